# revision 1
# baseline (speedup 1.0000x reference)
"""Multi-head attention (B=2, N=2048, D=1024, H=16) on 8 Trainium2 cores.

Sharding: data-parallel over batch (cores 0-3 -> b=0, cores 4-7 -> b=1) and
tensor-parallel over heads (4 heads per core = 256 of 1024 QKV/O channels).
Each core computes its 4 heads' attention plus a partial output projection;
the host sums the 4 partials per batch and adds bo.

Pipeline design (per core):
 - Q^T/K^T projections produce transposed layouts directly (x is
   pre-transposed on the host); V is projected straight into its natural
   [token, channel] layout with a ones column appended per head.
 - Attention runs one head at a time: scores S^T[k,q] (bf16, K=64 with
   tile_position row packing), exp(scale*s) fused on ScalarE reading PSUM
   (FD=1024), PV accumulates attn^T plus a softmax-denominator row (from
   the ones column of V).  The whole phase is ScalarE(exp)-bound.
 - Normalization: DVE reciprocal_approx_fast + GPSIMD partition_broadcast
   + DVE multiply.
 - PSUM banks: s1(2) s2(2) pv(2) pjA(1) pjB(1); projection and
   output-projection groups run on the 1-bank pj tags as paced "filler"
   work inside the attention k-loops so the PE overlaps the ScalarE phase.
   Fillers must be EMITTED before their consumers (Tile tracks dependencies
   in emission order) - the pacing below guarantees the required leads.
"""

import numpy as np

import concourse.bass as bass
import concourse.bacc as bacc
import concourse.tile as tile
from concourse import mybir
from concourse.bass_utils import run_bass_kernel_spmd

F32 = mybir.dt.float32
F32R = mybir.dt.float32r
BF16 = mybir.dt.bfloat16
AF = mybir.ActivationFunctionType

B, N, D, H, HD = 2, 2048, 1024, 16, 64
E = 256            # channels per core (4 heads * 64)
DC = D // 128      # 8 contraction chunks for projections
NB = N // 128      # 16 token blocks / k chunks
SCALE = 1.0 / np.sqrt(HD)
DT_PR = BF16       # projection matmul operands (x, Wq/Wk/Wv)
DT_SC = BF16       # scores matmul operands (qt/kt)
DT_PV = BF16       # PV matmul operands (vp, w=exp out)
DT_AT = BF16       # output-projection operands (attnT, WoT)


def _emit(nc):
    xT = nc.dram_tensor("xT", [D, N], DT_PR, kind="ExternalInput")
    wqT = nc.dram_tensor("wqT", [D, E], DT_PR, kind="ExternalInput")
    wkT = nc.dram_tensor("wkT", [D, E], DT_PR, kind="ExternalInput")
    wvT = nc.dram_tensor("wvT", [D, E], DT_PR, kind="ExternalInput")
    woT = nc.dram_tensor("woT", [E, D], DT_AT, kind="ExternalInput")
    bq2 = nc.dram_tensor("bq2", [128, 2], F32, kind="ExternalInput")
    bk2 = nc.dram_tensor("bk2", [128, 2], F32, kind="ExternalInput")
    bv1 = nc.dram_tensor("bv1", [E], F32, kind="ExternalInput")
    vones = nc.dram_tensor("vones", [128, NB, 4], DT_PV, kind="ExternalInput")
    out = nc.dram_tensor("out", [N, D], F32, kind="ExternalOutput")

    with tile.TileContext(nc) as tc:
        with tc.tile_pool(name="per", bufs=1) as per, \
             tc.tile_pool(name="wp", bufs=12) as wp, \
             tc.tile_pool(name="dn", bufs=2) as dn, \
             tc.tile_pool(name="up", bufs=2) as up, \
             tc.tile_pool(name="op", bufs=4) as op, \
             tc.tile_pool(name="ps", bufs=1, space="PSUM") as ps:

            # ---- persistent SBUF tiles ----
            xt = per.tile([128, DC, N], DT_PR)           # x[b].T (d-chunk, tokens)
            wq = per.tile([128, DC, E], DT_PR)
            wk = per.tile([128, DC, E], DT_PR)
            wv = per.tile([128, DC, E], DT_PR)
            wo = per.tile([128, 2, D], DT_AT)            # WoT (e-chunk)
            qt = per.tile([128, 2, N], DT_SC)            # Q^T: (pair, head-half)
            kt = per.tile([128, 2, N], DT_SC)
            vp = per.tile([128, NB, 4, 128], DT_PV)      # V natural + ones col (256B-aligned head stride for xbar transpose)
            at = per.tile([128, 2, N], DT_AT)            # attn^T normalized
            bqs = per.tile([128, 2], F32)
            bks = per.tile([128, 2], F32)
            bvb = per.tile([128, E], F32)

            for dc in range(DC):
                nc.sync.dma_start(out=xt[:, dc, :], in_=xT[dc * 128:(dc + 1) * 128, :])
                nc.sync.dma_start(out=wq[:, dc, :], in_=wqT[dc * 128:(dc + 1) * 128, :])
                nc.sync.dma_start(out=wk[:, dc, :], in_=wkT[dc * 128:(dc + 1) * 128, :])
                nc.sync.dma_start(out=wv[:, dc, :], in_=wvT[dc * 128:(dc + 1) * 128, :])
            for ec in range(2):
                nc.sync.dma_start(out=wo[:, ec, :], in_=woT[ec * 128:(ec + 1) * 128, :])
            nc.sync.dma_start(out=bqs, in_=bq2[:, :])
            nc.sync.dma_start(out=bks, in_=bk2[:, :])
            bv_ap = bv1[:]
            nc.gpsimd.dma_start(
                out=bvb,
                in_=bass.AP(tensor=bv_ap.tensor, offset=0, ap=[[0, 128], [1, E]]),
            )
            nc.sync.dma_start(out=vp[:, :, :, HD:HD + 1],
                              in_=vones[:, :, :].rearrange("p a (b o) -> p a b o", o=1))

            # PE warm-up: ~4us of dummy matmuls so the HAM un-throttles the
            # PE clock before the first projection groups arrive.
            wu = per.tile([64, 512], DT_PR)
            nc.vector.memset(wu, 0.0)
            wps = ps.tile([64, 512], F32, tag="pjA", name="wps")
            for i in range(10):
                nc.tensor.matmul(wps[:, :], wu[:, 0:64], wu[:, :],
                                 start=True, stop=True)

            pj_n = [0]

            def pj_tag():
                pj_n[0] += 1
                return ("pjA", "pjB")[pj_n[0] % 2]

            # ---- filler units (each: one 1-bank psum group on a pj tag) ----
            def proj_group(wsb, dst, bias, pair, n4):
                def emit():
                    pt = ps.tile([128, 512], F32, tag=pj_tag(), name="ppj")
                    for dc in range(DC):
                        nc.tensor.matmul(
                            pt[:, :],
                            wsb[:, dc, pair * 128:(pair + 1) * 128],
                            xt[:, dc, n4 * 512:(n4 + 1) * 512],
                            start=(dc == 0), stop=(dc == DC - 1),
                        )
                    nc.vector.tensor_scalar_add(
                        dst[:, pair, n4 * 512:(n4 + 1) * 512], pt[:, :],
                        bias[:, pair:pair + 1],
                    )
                return emit

            def vnat_group(nb):
                def emit():
                    pt = ps.tile([128, E], F32, tag=pj_tag(), name="pvn")
                    for dc in range(DC):
                        nc.tensor.matmul(
                            pt[:, :],
                            xt[:, dc, nb * 128:(nb + 1) * 128],
                            wv[:, dc, :],
                            start=(dc == 0), stop=(dc == DC - 1),
                        )
                    nc.vector.tensor_add(
                        vp[:, nb, :, 0:HD],
                        pt.rearrange("p (h d) -> p h d", h=4),
                        bvb.rearrange("p (h d) -> p h d", h=4),
                    )
                return emit

            def oproj_unit(nb, half, evict="dve"):
                def emit():
                    po = ps.tile([128, 512], F32, tag=pj_tag(), name="po")
                    for ec in range(2):
                        nc.tensor.matmul(
                            po[:, :],
                            at[:, ec, nb * 128:(nb + 1) * 128],
                            wo[:, ec, half * 512:(half + 1) * 512],
                            start=(ec == 0), stop=(ec == 1),
                        )
                    ot = op.tile([128, 512], F32, tag="ot", name="ot")
                    if evict == "dve":
                        nc.vector.tensor_copy(ot, po)
                    else:
                        nc.scalar.copy(ot, po)
                    nc.sync.dma_start(
                        out=out[nb * 128:(nb + 1) * 128,
                                half * 512:(half + 1) * 512],
                        in_=ot)
                return emit

            # ---- attention for one head (16 k-iters, s1/s2 double buffer) ----
            def attn_head(pair, q2, hh, fillers, prio_off=-160):
                q0 = q2 * 1024
                p0 = hh * 64
                fi = 0
                pv = ps.tile([HD + 1, 1024], F32, tag="pv", name="pv")
                wtiles = {}
                for k in range(NB):
                    while fi < (k + 1) * len(fillers) // NB:
                        with tc.high_priority(offset=prio_off):
                            fillers[fi]()
                        fi += 1
                    st = ps.tile([128, 1024], F32, tag=("s1", "s2")[k % 2],
                                 name="st")
                    for half in range(2):
                        nc.tensor.matmul(
                            st[:, half * 512:(half + 1) * 512],
                            kt[p0:p0 + 64, pair, k * 128:(k + 1) * 128],
                            qt[p0:p0 + 64, pair,
                               q0 + half * 512:q0 + (half + 1) * 512],
                            start=True, stop=True,
                            tile_position=(p0, 0),
                        )
                    w = wp.tile([128, 1024], DT_PV, tag="w", name="w")
                    nc.scalar.activation(w, st, AF.Exp, scale=SCALE)
                    wtiles[k] = w
                    if k > 0:
                        wprev = wtiles.pop(k - 1)
                        for half in range(2):
                            nc.tensor.matmul(
                                pv[:, half * 512:(half + 1) * 512],
                                vp[:, k - 1, 2 * pair + hh, 0:HD + 1],
                                wprev[:, half * 512:(half + 1) * 512],
                                start=(k - 1 == 0), stop=False,
                            )
                wlast = wtiles.pop(NB - 1)
                for half in range(2):
                    nc.tensor.matmul(
                        pv[:, half * 512:(half + 1) * 512],
                        vp[:, NB - 1, 2 * pair + hh, 0:HD + 1],
                        wlast[:, half * 512:(half + 1) * 512],
                        start=False, stop=True,
                    )
                while fi < len(fillers):
                    with tc.high_priority(offset=prio_off):
                        fillers[fi]()
                    fi += 1
                # normalize: attn^T[d, q] / den[q]
                den = dn.tile([1, 1024], F32, tag="den", name="den")
                rec = dn.tile([1, 1024], F32, tag="rec", name="rec")
                bcr = up.tile([HD, 1024], F32, tag="bcr", name="bcr")
                u = up.tile([HD, 1024], F32, tag="u", name="u")
                nc.vector.tensor_copy(den, pv[HD:HD + 1, :])
                nc.vector.tensor_copy(u, pv[0:HD, :])
                nc.vector.reciprocal_approx_fast(rec, den)
                nc.gpsimd.partition_broadcast(bcr, rec[0:1, :])
                nc.vector.tensor_mul(
                    at[p0:p0 + 64, pair, q0:q0 + 1024], u, bcr)

            # ---- emission schedule ----
            # NOTE: consumers must be EMITTED after their producers (Tile
            # tracks dependencies in emission order), so projection fillers
            # are placed with enough lead before the k-iters that read them.
            K0 = [proj_group(wk, kt, bks, 0, i) for i in range(4)]
            Q0 = [proj_group(wq, qt, bqs, 0, i) for i in range(4)]
            K1 = [proj_group(wk, kt, bks, 1, i) for i in range(4)]
            Q1 = [proj_group(wq, qt, bqs, 1, i) for i in range(4)]
            V = [vnat_group(i) for i in range(NB)]
            O0 = [oproj_unit(nb, h) for nb in range(8) for h in range(2)]
            O1 = [oproj_unit(nb, h, evict=("dve", "act")[(nb + h) % 2])
                  for nb in range(8, 16) for h in range(2)]

            for g in (K0[0], Q0[0], Q0[1], V[0], V[1]):
                g()
            attn_head(0, 0, 0, [V[2], V[3], K0[1], V[4], V[5], V[6], V[7],
                                K0[2], V[8], V[9], V[10], V[11], K0[3],
                                V[12], V[13], V[14], V[15]], prio_off=0)
            attn_head(0, 0, 1, [K1[0], K1[1], K1[2], K1[3],
                                Q1[0], Q1[1], Q1[2], Q1[3]], prio_off=0)
            attn_head(1, 0, 0, [Q0[2], Q0[3]], prio_off=0)
            attn_head(1, 0, 1, [])
            attn_head(0, 1, 0, O0[0:4])
            attn_head(0, 1, 1, O0[4:8])
            attn_head(1, 1, 0, O0[8:12])
            attn_head(1, 1, 1, O0[12:16])
            for g in O1:
                g()
    return nc


_CACHE = {}


def _build():
    if "nc" not in _CACHE:
        nc = bacc.Bacc("TRN2", target_bir_lowering=False, debug=False)
        _emit(nc)
        nc.compile()
        _CACHE["nc"] = nc
    return _CACHE["nc"]


def make_in_maps(x, Wq, bq, Wk, bk, Wv, bv, Wo, bo):
    import ml_dtypes
    f32 = np.float32
    bt = ml_dtypes.bfloat16
    dpr = bt if DT_PR == BF16 else f32
    ones_np = np.ones((128, NB, 4), bt if DT_PV == BF16 else f32)
    xTs = [np.ascontiguousarray(np.asarray(x[b], dtype=f32).T).astype(dpr)
           for b in range(B)]
    in_maps = []
    for c in range(8):
        b, r0 = c // 4, (c % 4) * E
        rows = slice(r0, r0 + E)
        in_maps.append({
            "xT": xTs[b],
            "wqT": np.ascontiguousarray(np.asarray(Wq, f32)[rows].T).astype(dpr),
            "wkT": np.ascontiguousarray(np.asarray(Wk, f32)[rows].T).astype(dpr),
            "wvT": np.ascontiguousarray(np.asarray(Wv, f32)[rows].T).astype(dpr),
            "woT": np.ascontiguousarray(
                np.asarray(Wo, f32)[:, rows].T).astype(bt if DT_AT == BF16 else f32),
            "bq2": np.ascontiguousarray(np.asarray(bq, f32)[rows].reshape(2, 128).T),
            "bk2": np.ascontiguousarray(np.asarray(bk, f32)[rows].reshape(2, 128).T),
            "bv1": np.ascontiguousarray(np.asarray(bv, f32)[rows]),
            "vones": ones_np,
        })
    return in_maps


def kernel(x, Wq, bq, Wk, bk, Wv, bv, Wo, bo, _spmd_kwargs=None):
    nc = _build()
    in_maps = make_in_maps(x, Wq, bq, Wk, bk, Wv, bv, Wo, bo)
    res = run_bass_kernel_spmd(nc, in_maps, core_ids=list(range(8)),
                               **(_spmd_kwargs or {}))
    parts = np.stack([res.results[c]["out"] for c in range(8)])
    outv = parts.reshape(B, 4, N, D).sum(axis=1) + np.asarray(bo, np.float32)
    if _spmd_kwargs:
        _CACHE["last_results"] = res
    return outv.astype(np.float32)



# revision 4
# speedup vs baseline: 1.1247x; 1.1247x over previous
"""Multi-head attention (B=2, N=2048, D=1024, H=16) on 8 Trainium2 cores.

Sharding: data-parallel over batch (cores 0-3 -> b=0, cores 4-7 -> b=1) and
tensor-parallel over heads (4 heads per core = 256 of 1024 QKV/O channels).
Each core computes its 4 heads' attention plus a partial output projection;
the host sums the 4 partials per batch and adds bo.

v2 pipeline (per core):
 - Input DMA spread over 4 engine queues; projections run chunk-major in a
   dedicated pre-phase PSUM pool so each weight-chunk matmul fires as soon
   as its xT d-chunk lands.
 - Attention processes a (pair, 512-query-chunk) block at a time.  Per
   k-iter the TWO heads of the pair run their scores matmuls CONCURRENTLY
   in disjoint PE row-groups (K=64 each, tile_position (0,0)/(64,0)) into
   the two halves of one [128,1024] PSUM tile; a single FD=1024 exp on
   ScalarE covers both heads; PV (M=65 with the ones/denominator column)
   runs per head with a one-iter lag like the baseline.
 - Normalization reads PSUM directly: reciprocal_approx_fast on the den
   row, GPSIMD partition_broadcast, one tensor_mul into at.
 - qc-outer / pair-inner block order lets oproj units for query chunk qc
   run as PE filler work two blocks later; only the last chunk's oproj
   trails the attention.
"""

import numpy as np

import concourse.bass as bass
import concourse.bacc as bacc
import concourse.tile as tile
from concourse import mybir
from concourse.bass_utils import run_bass_kernel_spmd

F32 = mybir.dt.float32
BF16 = mybir.dt.bfloat16
AF = mybir.ActivationFunctionType

B, N, D, H, HD = 2, 2048, 1024, 16, 64
E = 256            # channels per core (4 heads * 64)
DC = D // 128      # 8 contraction chunks for projections
NB = N // 128      # 16 token blocks / k chunks
QC = 512           # query chunk
NQC = N // QC      # 4 query chunks
SCALE = 1.0 / np.sqrt(HD)
DT = BF16


def _emit(nc):
    xT = nc.dram_tensor("xT", [D, N], DT, kind="ExternalInput")
    wqT = nc.dram_tensor("wqT", [D, E], DT, kind="ExternalInput")
    wkT = nc.dram_tensor("wkT", [D, E], DT, kind="ExternalInput")
    wvT = nc.dram_tensor("wvT", [D, E], DT, kind="ExternalInput")
    woT = nc.dram_tensor("woT", [E, D], DT, kind="ExternalInput")
    bq2 = nc.dram_tensor("bq2", [128, 2], F32, kind="ExternalInput")
    bk2 = nc.dram_tensor("bk2", [128, 2], F32, kind="ExternalInput")
    bv1 = nc.dram_tensor("bv1", [E], F32, kind="ExternalInput")
    vones = nc.dram_tensor("vones", [128, NB, 4], DT, kind="ExternalInput")
    out = nc.dram_tensor("out", [N, D], F32, kind="ExternalOutput")

    with tile.TileContext(nc) as tc:
        with tc.tile_pool(name="per", bufs=1) as per, \
             tc.tile_pool(name="wp", bufs=12) as wp, \
             tc.tile_pool(name="dn", bufs=2) as dn, \
             tc.tile_pool(name="up", bufs=2) as up, \
             tc.tile_pool(name="op", bufs=4) as op:

            # ---- persistent SBUF tiles ----
            xt = per.tile([128, DC, N], DT)           # x[b].T (d-chunk, tokens)
            wq = per.tile([128, DC, E], DT)
            wk = per.tile([128, DC, E], DT)
            wv = per.tile([128, DC, E], DT)
            wo = per.tile([128, 2, D], DT)            # WoT (e-chunk)
            qt = per.tile([128, 2, N], DT)            # Q^T: (pair, tokens)
            kt = per.tile([128, 2, N], DT)
            vp = per.tile([128, NB, 4, 128], DT)      # V natural + ones col
            at = per.tile([128, 2, N], DT)            # attn^T normalized
            bqs = per.tile([128, 2], F32)
            bks = per.tile([128, 2], F32)
            bvb = per.tile([128, E], F32)

            # ---- input DMA spread over 3 queues ----
            qs = [nc.sync, nc.scalar, nc.gpsimd]
            for dc in range(DC):
                qs[dc % 3].dma_start(out=xt[:, dc, :],
                                     in_=xT[dc * 128:(dc + 1) * 128, :])
            for dc in range(DC):
                qs[(dc + 1) % 3].dma_start(out=wk[:, dc, :],
                                           in_=wkT[dc * 128:(dc + 1) * 128, :])
                qs[(dc + 2) % 3].dma_start(out=wq[:, dc, :],
                                           in_=wqT[dc * 128:(dc + 1) * 128, :])
                qs[dc % 3].dma_start(out=wv[:, dc, :],
                                     in_=wvT[dc * 128:(dc + 1) * 128, :])
            nc.sync.dma_start(out=bqs, in_=bq2[:, :])
            nc.sync.dma_start(out=bks, in_=bk2[:, :])
            bv_ap = bv1[:]
            nc.gpsimd.dma_start(
                out=bvb,
                in_=bass.AP(tensor=bv_ap.tensor, offset=0, ap=[[0, 128], [1, E]]),
            )
            nc.scalar.dma_start(out=vp[:, :, :, HD:HD + 1],
                                in_=vones[:, :, :].rearrange(
                                    "p a (b o) -> p a b o", o=1))
            for ec in range(2):
                nc.gpsimd.dma_start(out=wo[:, ec, :],
                                    in_=woT[ec * 128:(ec + 1) * 128, :])

            # ---- pre-phase: warmup + chunk-major first projections ----
            # K0 g0-3, Q0 g0, V nb0-1 accumulate concurrently in a dedicated
            # PSUM pool (7 banks); each d-chunk's matmuls fire as the chunk
            # arrives from HBM.
            with tc.tile_pool(name="pre", bufs=1, space="PSUM") as pre:
                wu = per.tile([64, 512], DT)
                nc.vector.memset(wu, 0.0)
                wps = pre.tile([64, 512], F32, tag="p7", name="wps")
                for i in range(10):
                    nc.tensor.matmul(wps[:, :], wu[:, 0:64], wu[:, :],
                                     start=True, stop=True)

                pk = [pre.tile([128, 512], F32, tag=f"p{g}", name=f"pk{g}")
                      for g in range(4)]
                pq0 = pre.tile([128, 512], F32, tag="p4", name="pq0")
                pv01 = [pre.tile([128, E], F32, tag=f"p{5 + i}", name=f"pv{i}")
                        for i in range(2)]
                for dc in range(DC):
                    for g in range(4):
                        nc.tensor.matmul(
                            pk[g], wk[:, dc, 0:128],
                            xt[:, dc, g * 512:(g + 1) * 512],
                            start=(dc == 0), stop=(dc == DC - 1))
                    nc.tensor.matmul(
                        pq0, wq[:, dc, 0:128], xt[:, dc, 0:512],
                        start=(dc == 0), stop=(dc == DC - 1))
                    for i in range(2):
                        nc.tensor.matmul(
                            pv01[i], xt[:, dc, i * 128:(i + 1) * 128],
                            wv[:, dc, :],
                            start=(dc == 0), stop=(dc == DC - 1))
                for g in range(4):
                    nc.vector.tensor_scalar_add(
                        kt[:, 0, g * 512:(g + 1) * 512], pk[g], bks[:, 0:1])
                nc.vector.tensor_scalar_add(qt[:, 0, 0:512], pq0, bqs[:, 0:1])
                for i in range(2):
                    nc.vector.tensor_add(
                        vp[:, i, :, 0:HD],
                        pv01[i].rearrange("p (h d) -> p h d", h=4),
                        bvb.rearrange("p (h d) -> p h d", h=4))

            with tc.tile_pool(name="ps", bufs=1, space="PSUM") as ps:
                pj_n = [0]

                def pj_tag():
                    pj_n[0] += 1
                    return ("pjA", "pjB")[pj_n[0] % 2]

                # ---- filler units (1-bank psum groups on pj tags) ----
                def proj_group(wsb, dst, bias, pair, n4):
                    def emit():
                        pt = ps.tile([128, 512], F32, tag=pj_tag(), name="ppj")
                        for dc in range(DC):
                            nc.tensor.matmul(
                                pt[:, :],
                                wsb[:, dc, pair * 128:(pair + 1) * 128],
                                xt[:, dc, n4 * 512:(n4 + 1) * 512],
                                start=(dc == 0), stop=(dc == DC - 1))
                        nc.vector.tensor_scalar_add(
                            dst[:, pair, n4 * 512:(n4 + 1) * 512], pt[:, :],
                            bias[:, pair:pair + 1])
                    return emit

                def vnat_group(nb):
                    def emit():
                        pt = ps.tile([128, E], F32, tag=pj_tag(), name="pvn")
                        for dc in range(DC):
                            nc.tensor.matmul(
                                pt[:, :],
                                xt[:, dc, nb * 128:(nb + 1) * 128],
                                wv[:, dc, :],
                                start=(dc == 0), stop=(dc == DC - 1))
                        nc.vector.tensor_add(
                            vp[:, nb, :, 0:HD],
                            pt.rearrange("p (h d) -> p h d", h=4),
                            bvb.rearrange("p (h d) -> p h d", h=4))
                    return emit

                def oproj_unit(nb, half, evict="dve"):
                    def emit():
                        po = ps.tile([128, 512], F32, tag=pj_tag(), name="po")
                        for ec in range(2):
                            nc.tensor.matmul(
                                po[:, :],
                                at[:, ec, nb * 128:(nb + 1) * 128],
                                wo[:, ec, half * 512:(half + 1) * 512],
                                start=(ec == 0), stop=(ec == 1))
                        ot = op.tile([128, 512], F32, tag="ot", name="ot")
                        if evict == "dve":
                            nc.vector.tensor_copy(ot, po)
                        else:
                            nc.scalar.copy(ot, po)
                        nc.sync.dma_start(
                            out=out[nb * 128:(nb + 1) * 128,
                                    half * 512:(half + 1) * 512],
                            in_=ot)
                    return emit

                # ---- one (pair, qc) attention block: 16 k-iters ----
                def attn_block(pair, qc, fillers, prio_off=0):
                    q0 = qc * QC
                    fi = 0
                    pvs = [ps.tile([HD + 1, QC], F32, tag=t, name=t)
                           for t in ("pvA", "pvB")]
                    wtiles = {}
                    for k in range(NB):
                        while fi < (k + 1) * len(fillers) // NB:
                            with tc.high_priority(offset=prio_off):
                                fillers[fi]()
                            fi += 1
                        st = ps.tile([128, 1024], F32,
                                     tag=("s0", "s1")[k % 2], name="st")
                        for hh in range(2):
                            p0 = hh * HD
                            nc.tensor.matmul(
                                st[:, hh * QC:(hh + 1) * QC],
                                kt[p0:p0 + HD, pair, k * 128:(k + 1) * 128],
                                qt[p0:p0 + HD, pair, q0:q0 + QC],
                                start=True, stop=True,
                                tile_position=(p0, 0))
                        w = wp.tile([128, 1024], DT, tag="w", name="w")
                        nc.scalar.activation(w, st, AF.Exp, scale=SCALE)
                        wtiles[k] = w
                        if k > 0:
                            wprev = wtiles.pop(k - 1)
                            for hh in range(2):
                                nc.tensor.matmul(
                                    pvs[hh][:, :],
                                    vp[:, k - 1, 2 * pair + hh, 0:HD + 1],
                                    wprev[:, hh * QC:(hh + 1) * QC],
                                    start=(k - 1 == 0), stop=False)
                    wlast = wtiles.pop(NB - 1)
                    for hh in range(2):
                        nc.tensor.matmul(
                            pvs[hh][:, :],
                            vp[:, NB - 1, 2 * pair + hh, 0:HD + 1],
                            wlast[:, hh * QC:(hh + 1) * QC],
                            start=False, stop=True)
                    while fi < len(fillers):
                        with tc.high_priority(offset=prio_off):
                            fillers[fi]()
                        fi += 1
                    # normalize both heads (den/u staged through SBUF)
                    for hh in range(2):
                        p0 = hh * HD
                        den = dn.tile([1, QC], F32, tag="den", name="den")
                        rec = dn.tile([1, QC], F32, tag="rec", name="rec")
                        bcr = up.tile([HD, QC], F32, tag="bcr", name="bcr")
                        u = up.tile([HD, QC], F32, tag="u", name="u")
                        nc.vector.tensor_copy(den, pvs[hh][HD:HD + 1, :])
                        nc.vector.tensor_copy(u, pvs[hh][0:HD, :])
                        nc.vector.reciprocal_approx_fast(rec, den)
                        nc.gpsimd.partition_broadcast(bcr, rec[0:1, :])
                        nc.vector.tensor_mul(
                            at[p0:p0 + HD, pair, q0:q0 + QC], u, bcr)

                # ---- filler inventory ----
                V = [vnat_group(i) for i in range(NB)]
                K1 = [proj_group(wk, kt, bks, 1, g) for g in range(4)]
                Q0 = [proj_group(wq, qt, bqs, 0, g) for g in range(4)]
                Q1 = [proj_group(wq, qt, bqs, 1, g) for g in range(4)]
                O = [[oproj_unit(4 * qc + nb, h) for nb in range(4)
                      for h in range(2)] for qc in range(NQC)]

                # V[nb] feeds PV at iter nb of the qc=0 blocks; keep a
                # 2-iter lead.  K1 g must land before block (1,0) iter 4g.
                attn_block(0, 0, [V[2], V[3], V[4], V[5], V[6], V[7],
                                  V[8], V[9], V[10], V[11], V[12], V[13],
                                  V[14], V[15], K1[0], Q1[0]])
                attn_block(1, 0, [K1[1], K1[2], K1[3], Q0[1]])
                attn_block(0, 1, [Q1[1]] + O[0][0:4])
                attn_block(1, 1, O[0][4:8] + [Q0[2]])
                attn_block(0, 2, [Q1[2]] + O[1][0:4])
                attn_block(1, 2, O[1][4:8] + [Q0[3]])
                attn_block(0, 3, [Q1[3]] + O[2][0:4])
                attn_block(1, 3, O[2][4:8])
                for g in O[3]:
                    g()
    return nc


_CACHE = {}


def _build():
    if "nc" not in _CACHE:
        nc = bacc.Bacc("TRN2", target_bir_lowering=False, debug=False)
        _emit(nc)
        nc.compile()
        _CACHE["nc"] = nc
    return _CACHE["nc"]


def make_in_maps(x, Wq, bq, Wk, bk, Wv, bv, Wo, bo):
    import ml_dtypes
    f32 = np.float32
    bt = ml_dtypes.bfloat16
    ones_np = np.ones((128, NB, 4), bt)
    xTs = [np.ascontiguousarray(np.asarray(x[b], dtype=f32).T).astype(bt)
           for b in range(B)]
    in_maps = []
    for c in range(8):
        b, r0 = c // 4, (c % 4) * E
        rows = slice(r0, r0 + E)
        in_maps.append({
            "xT": xTs[b],
            "wqT": np.ascontiguousarray(np.asarray(Wq, f32)[rows].T).astype(bt),
            "wkT": np.ascontiguousarray(np.asarray(Wk, f32)[rows].T).astype(bt),
            "wvT": np.ascontiguousarray(np.asarray(Wv, f32)[rows].T).astype(bt),
            "woT": np.ascontiguousarray(np.asarray(Wo, f32)[:, rows].T).astype(bt),
            "bq2": np.ascontiguousarray(np.asarray(bq, f32)[rows].reshape(2, 128).T),
            "bk2": np.ascontiguousarray(np.asarray(bk, f32)[rows].reshape(2, 128).T),
            "bv1": np.ascontiguousarray(np.asarray(bv, f32)[rows]),
            "vones": ones_np,
        })
    return in_maps


def kernel(x, Wq, bq, Wk, bk, Wv, bv, Wo, bo, _spmd_kwargs=None):
    nc = _build()
    in_maps = make_in_maps(x, Wq, bq, Wk, bk, Wv, bv, Wo, bo)
    res = run_bass_kernel_spmd(nc, in_maps, core_ids=list(range(8)),
                               **(_spmd_kwargs or {}))
    parts = np.stack([res.results[c]["out"] for c in range(8)])
    outv = parts.reshape(B, 4, N, D).sum(axis=1) + np.asarray(bo, np.float32)
    if _spmd_kwargs:
        _CACHE["last_results"] = res
    return outv.astype(np.float32)


# revision 8
# speedup vs baseline: 1.1868x; 1.0553x over previous
"""Multi-head attention (B=2, N=2048, D=1024, H=16) on 8 Trainium2 cores.

Sharding: data-parallel over batch (cores 0-3 -> b=0, cores 4-7 -> b=1) and
tensor-parallel over heads (4 heads per core = 256 of 1024 QKV/O channels).
Each core computes its 4 heads' attention plus a partial output projection;
the host sums the 4 partials per batch and adds bo.

v2 pipeline (per core):
 - Input DMA spread over 4 engine queues; projections run chunk-major in a
   dedicated pre-phase PSUM pool so each weight-chunk matmul fires as soon
   as its xT d-chunk lands.
 - Attention processes a (pair, 512-query-chunk) block at a time.  Per
   k-iter the TWO heads of the pair run their scores matmuls CONCURRENTLY
   in disjoint PE row-groups (K=64 each, tile_position (0,0)/(64,0)) into
   the two halves of one [128,1024] PSUM tile; a single FD=1024 exp on
   ScalarE covers both heads; PV (M=65 with the ones/denominator column)
   runs per head with a one-iter lag like the baseline.
 - Normalization reads PSUM directly: reciprocal_approx_fast on the den
   row, GPSIMD partition_broadcast, one tensor_mul into at.
 - qc-outer / pair-inner block order lets oproj units for query chunk qc
   run as PE filler work two blocks later; only the last chunk's oproj
   trails the attention.
"""

import numpy as np

import concourse.bass as bass
import concourse.bacc as bacc
import concourse.tile as tile
from concourse import mybir
from concourse.bass_utils import run_bass_kernel_spmd

F32 = mybir.dt.float32
BF16 = mybir.dt.bfloat16
AF = mybir.ActivationFunctionType

B, N, D, H, HD = 2, 2048, 1024, 16, 64
E = 256            # channels per core (4 heads * 64)
DC = D // 128      # 8 contraction chunks for projections
NB = N // 128      # 16 token blocks / k chunks
QC = 512           # query chunk
NQC = N // QC      # 4 query chunks
SCALE = 1.0 / np.sqrt(HD)
DT = BF16


def _emit(nc):
    xT = nc.dram_tensor("xT", [D, N], DT, kind="ExternalInput")
    wqT = nc.dram_tensor("wqT", [D, E], DT, kind="ExternalInput")
    wkT = nc.dram_tensor("wkT", [D, E], DT, kind="ExternalInput")
    wvT = nc.dram_tensor("wvT", [D, E], DT, kind="ExternalInput")
    woT = nc.dram_tensor("woT", [E, D], DT, kind="ExternalInput")
    bq2 = nc.dram_tensor("bq2", [128, 2], F32, kind="ExternalInput")
    bk2 = nc.dram_tensor("bk2", [128, 2], F32, kind="ExternalInput")
    bv1 = nc.dram_tensor("bv1", [E], F32, kind="ExternalInput")
    vones = nc.dram_tensor("vones", [128, NB, 4], DT, kind="ExternalInput")
    out = nc.dram_tensor("out", [N, D], F32, kind="ExternalOutput")

    with tile.TileContext(nc) as tc:
        with tc.tile_pool(name="per", bufs=1) as per, \
             tc.tile_pool(name="wp", bufs=12) as wp, \
             tc.tile_pool(name="dn", bufs=2) as dn, \
             tc.tile_pool(name="up", bufs=2) as up, \
             tc.tile_pool(name="op", bufs=4) as op:

            # ---- persistent SBUF tiles ----
            xt = per.tile([128, DC, N], DT)           # x[b].T (d-chunk, tokens)
            wq = per.tile([128, DC, E], DT)
            wk = per.tile([128, DC, E], DT)
            wv = per.tile([128, DC, E], DT)
            wo = per.tile([128, 2, D], DT)            # WoT (e-chunk)
            qt = per.tile([128, 2, N], DT)            # Q^T: (pair, tokens)
            kt = per.tile([128, 2, N], DT)
            vp = per.tile([128, NB, 4, 128], DT)      # V natural + ones col
            at = per.tile([128, 2, N], DT)            # attn^T normalized
            bqs = per.tile([128, 2], F32)
            bks = per.tile([128, 2], F32)
            bvb = per.tile([128, E], F32)

            # ---- input DMA: per-d-chunk rounds over 3 queues ----
            # Each chunk's small W slices load ahead of its big xT slice on
            # the same queue so the chunk-major pre-phase can start as soon
            # as a whole d-chunk has landed.
            qs = [nc.sync, nc.scalar, nc.gpsimd]
            for dc in range(DC):
                q = qs[dc % 3]
                q.dma_start(out=wk[:, dc, :], in_=wkT[dc * 128:(dc + 1) * 128, :])
                q.dma_start(out=wq[:, dc, :], in_=wqT[dc * 128:(dc + 1) * 128, :])
                q.dma_start(out=wv[:, dc, :], in_=wvT[dc * 128:(dc + 1) * 128, :])
                q.dma_start(out=xt[:, dc, :], in_=xT[dc * 128:(dc + 1) * 128, :])
            nc.sync.dma_start(out=bqs, in_=bq2[:, :])
            nc.sync.dma_start(out=bks, in_=bk2[:, :])
            bv_ap = bv1[:]
            nc.gpsimd.dma_start(
                out=bvb,
                in_=bass.AP(tensor=bv_ap.tensor, offset=0, ap=[[0, 128], [1, E]]),
            )
            nc.scalar.dma_start(out=vp[:, :, :, HD:HD + 1],
                                in_=vones[:, :, :].rearrange(
                                    "p a (b o) -> p a b o", o=1))
            for ec in range(2):
                nc.gpsimd.dma_start(out=wo[:, ec, :],
                                    in_=woT[ec * 128:(ec + 1) * 128, :])

            # ---- pre-phase: warmup + chunk-major first projections ----
            # K0 g0-3, Q0 g0, V nb0-1 accumulate concurrently in a dedicated
            # PSUM pool (7 banks); each d-chunk's matmuls fire as the chunk
            # arrives from HBM.
            with tc.tile_pool(name="pre", bufs=1, space="PSUM") as pre:
                wu = per.tile([64, 512], DT)
                nc.vector.memset(wu, 0.0)
                wps = pre.tile([64, 512], F32, tag="p7", name="wps")
                for i in range(10):
                    nc.tensor.matmul(wps[:, :], wu[:, 0:64], wu[:, :],
                                     start=True, stop=True)

                pk = [pre.tile([128, 512], F32, tag=f"p{g}", name=f"pk{g}")
                      for g in range(4)]
                pq0 = pre.tile([128, 512], F32, tag="p4", name="pq0")
                pv01 = [pre.tile([128, E], F32, tag=f"p{5 + i}", name=f"pv{i}")
                        for i in range(2)]
                for dc in range(DC):
                    for g in range(4):
                        nc.tensor.matmul(
                            pk[g], wk[:, dc, 0:128],
                            xt[:, dc, g * 512:(g + 1) * 512],
                            start=(dc == 0), stop=(dc == DC - 1))
                    nc.tensor.matmul(
                        pq0, wq[:, dc, 0:128], xt[:, dc, 0:512],
                        start=(dc == 0), stop=(dc == DC - 1))
                    for i in range(2):
                        nc.tensor.matmul(
                            pv01[i], xt[:, dc, i * 128:(i + 1) * 128],
                            wv[:, dc, :],
                            start=(dc == 0), stop=(dc == DC - 1))
                for g in range(4):
                    nc.vector.tensor_scalar_add(
                        kt[:, 0, g * 512:(g + 1) * 512], pk[g], bks[:, 0:1])
                nc.vector.tensor_scalar_add(qt[:, 0, 0:512], pq0, bqs[:, 0:1])
                for i in range(2):
                    nc.vector.tensor_add(
                        vp[:, i, :, 0:HD],
                        pv01[i].rearrange("p (h d) -> p h d", h=4),
                        bvb.rearrange("p (h d) -> p h d", h=4))

            with tc.tile_pool(name="ps", bufs=1, space="PSUM") as ps:
                pj_n = [0]

                def pj_tag():
                    pj_n[0] += 1
                    return ("pjA", "pjB")[pj_n[0] % 2]

                # ---- filler units (1-bank psum groups on pj tags) ----
                def proj_group(wsb, dst, bias, pair, n4):
                    def emit():
                        pt = ps.tile([128, 512], F32, tag=pj_tag(), name="ppj")
                        for dc in range(DC):
                            nc.tensor.matmul(
                                pt[:, :],
                                wsb[:, dc, pair * 128:(pair + 1) * 128],
                                xt[:, dc, n4 * 512:(n4 + 1) * 512],
                                start=(dc == 0), stop=(dc == DC - 1))
                        nc.vector.tensor_scalar_add(
                            dst[:, pair, n4 * 512:(n4 + 1) * 512], pt[:, :],
                            bias[:, pair:pair + 1])
                    return emit

                def vnat_group(nb):
                    def emit():
                        pt = ps.tile([128, E], F32, tag=pj_tag(), name="pvn")
                        for dc in range(DC):
                            nc.tensor.matmul(
                                pt[:, :],
                                xt[:, dc, nb * 128:(nb + 1) * 128],
                                wv[:, dc, :],
                                start=(dc == 0), stop=(dc == DC - 1))
                        nc.vector.tensor_add(
                            vp[:, nb, :, 0:HD],
                            pt.rearrange("p (h d) -> p h d", h=4),
                            bvb.rearrange("p (h d) -> p h d", h=4))
                    return emit

                o_n = [0]

                def oproj_unit(nb, half, evict="dve"):
                    def emit():
                        po = ps.tile([128, 512], F32, tag=pj_tag(), name="po")
                        for ec in range(2):
                            nc.tensor.matmul(
                                po[:, :],
                                at[:, ec, nb * 128:(nb + 1) * 128],
                                wo[:, ec, half * 512:(half + 1) * 512],
                                start=(ec == 0), stop=(ec == 1))
                        ot = op.tile([128, 512], F32, tag="ot", name="ot")
                        if evict == "dve":
                            nc.vector.tensor_copy(ot, po)
                        else:
                            nc.scalar.copy(ot, po)
                        o_n[0] += 1
                        qs[o_n[0] % 3].dma_start(
                            out=out[nb * 128:(nb + 1) * 128,
                                    half * 512:(half + 1) * 512],
                            in_=ot)
                    return emit

                # ---- one (pair, qc) attention block: 16 k-iters ----
                # Returns a `finish` closure (last PV pair + normalization);
                # the caller runs it after the NEXT block's first k-iter so
                # ScalarE never stalls across block boundaries.
                def attn_block(pair, qc, fillers, carry=None):
                    q0 = qc * QC
                    fi = 0
                    pvs = [ps.tile([HD + 1, QC], F32, tag=t, name=t)
                           for t in ("pvA", "pvB")]
                    wtiles = {}
                    for k in range(NB):
                        st = ps.tile([128, 1024], F32,
                                     tag=("s0", "s1")[k % 2], name="st")
                        # scores for both heads back-to-back at max priority
                        # so they sit adjacent in the PE queue and overlap in
                        # disjoint row-groups of the array.
                        with tc.high_priority(offset=1 << 20):
                            for hh in range(2):
                                p0 = hh * HD
                                nc.tensor.matmul(
                                    st[:, hh * QC:(hh + 1) * QC],
                                    kt[p0:p0 + HD, pair, k * 128:(k + 1) * 128],
                                    qt[p0:p0 + HD, pair, q0:q0 + QC],
                                    start=True, stop=True,
                                    tile_position=(p0, 0))
                        w = wp.tile([128, 1024], DT, tag="w", name="w")
                        nc.scalar.activation(w, st, AF.Exp, scale=SCALE)
                        wtiles[k] = w
                        if k == 0 and carry is not None:
                            carry()
                            carry = None
                        while fi < (k + 1) * len(fillers) // NB:
                            fillers[fi]()
                            fi += 1
                        if k > 0:
                            wprev = wtiles.pop(k - 1)
                            for hh in range(2):
                                nc.tensor.matmul(
                                    pvs[hh][:, :],
                                    vp[:, k - 1, 2 * pair + hh, 0:HD + 1],
                                    wprev[:, hh * QC:(hh + 1) * QC],
                                    start=(k - 1 == 0), stop=False)
                    wlast = wtiles.pop(NB - 1)

                    def finish():
                        for hh in range(2):
                            nc.tensor.matmul(
                                pvs[hh][:, :],
                                vp[:, NB - 1, 2 * pair + hh, 0:HD + 1],
                                wlast[:, hh * QC:(hh + 1) * QC],
                                start=False, stop=True)
                        # normalize both heads (den/u staged through SBUF)
                        for hh in range(2):
                            p0 = hh * HD
                            den = dn.tile([1, QC], F32, tag="den", name="den")
                            rec = dn.tile([1, QC], F32, tag="rec", name="rec")
                            bcr = up.tile([HD, QC], F32, tag="bcr", name="bcr")
                            u = up.tile([HD, QC], F32, tag="u", name="u")
                            nc.vector.tensor_copy(den, pvs[hh][HD:HD + 1, :])
                            nc.vector.tensor_copy(u, pvs[hh][0:HD, :])
                            nc.vector.reciprocal_approx_fast(rec, den)
                            nc.gpsimd.partition_broadcast(bcr, rec[0:1, :])
                            nc.vector.tensor_mul(
                                at[p0:p0 + HD, pair, q0:q0 + QC], u, bcr)
                    return finish

                # ---- filler inventory ----
                V = [vnat_group(i) for i in range(NB)]
                K1 = [proj_group(wk, kt, bks, 1, g) for g in range(4)]
                Q0 = [proj_group(wq, qt, bqs, 0, g) for g in range(4)]
                Q1 = [proj_group(wq, qt, bqs, 1, g) for g in range(4)]
                O = [[oproj_unit(4 * qc + nb, h) for nb in range(4)
                      for h in range(2)] for qc in range(NQC)]

                # V[nb] feeds PV at iter nb of the qc=0 blocks; keep a
                # 2-iter lead.  K1 g must land before block (1,0) iter 4g.
                sched = [
                    (0, 0, [V[2], V[3], V[4], V[5], V[6], V[7],
                            V[8], V[9], V[10], V[11], V[12], V[13],
                            V[14], V[15], K1[0], Q1[0]]),
                    (1, 0, [K1[1], K1[2], K1[3], Q0[1]]),
                    (0, 1, [Q1[1]] + O[0][0:4]),
                    (1, 1, O[0][4:8] + [Q0[2]]),
                    (0, 2, [Q1[2]] + O[1][0:4]),
                    (1, 2, O[1][4:8] + [Q0[3]]),
                    (0, 3, [Q1[3]] + O[2][0:4]),
                    (1, 3, O[2][4:8]),
                ]
                fin = None
                for pair, qc, fillers in sched:
                    fin = attn_block(pair, qc, fillers, carry=fin)
                fin()
                for g in O[3]:
                    g()
    return nc


_CACHE = {}


def _build():
    if "nc" not in _CACHE:
        nc = bacc.Bacc("TRN2", target_bir_lowering=False, debug=False)
        _emit(nc)
        nc.compile()
        _CACHE["nc"] = nc
    return _CACHE["nc"]


def make_in_maps(x, Wq, bq, Wk, bk, Wv, bv, Wo, bo):
    import ml_dtypes
    f32 = np.float32
    bt = ml_dtypes.bfloat16
    ones_np = np.ones((128, NB, 4), bt)
    xTs = [np.ascontiguousarray(np.asarray(x[b], dtype=f32).T).astype(bt)
           for b in range(B)]
    in_maps = []
    for c in range(8):
        b, r0 = c // 4, (c % 4) * E
        rows = slice(r0, r0 + E)
        in_maps.append({
            "xT": xTs[b],
            "wqT": np.ascontiguousarray(np.asarray(Wq, f32)[rows].T).astype(bt),
            "wkT": np.ascontiguousarray(np.asarray(Wk, f32)[rows].T).astype(bt),
            "wvT": np.ascontiguousarray(np.asarray(Wv, f32)[rows].T).astype(bt),
            "woT": np.ascontiguousarray(np.asarray(Wo, f32)[:, rows].T).astype(bt),
            "bq2": np.ascontiguousarray(np.asarray(bq, f32)[rows].reshape(2, 128).T),
            "bk2": np.ascontiguousarray(np.asarray(bk, f32)[rows].reshape(2, 128).T),
            "bv1": np.ascontiguousarray(np.asarray(bv, f32)[rows]),
            "vones": ones_np,
        })
    return in_maps


def kernel(x, Wq, bq, Wk, bk, Wv, bv, Wo, bo, _spmd_kwargs=None):
    nc = _build()
    in_maps = make_in_maps(x, Wq, bq, Wk, bk, Wv, bv, Wo, bo)
    res = run_bass_kernel_spmd(nc, in_maps, core_ids=list(range(8)),
                               **(_spmd_kwargs or {}))
    parts = np.stack([res.results[c]["out"] for c in range(8)])
    outv = parts.reshape(B, 4, N, D).sum(axis=1) + np.asarray(bo, np.float32)
    if _spmd_kwargs:
        _CACHE["last_results"] = res
    return outv.astype(np.float32)


# revision 15
# speedup vs baseline: 1.1957x; 1.0075x over previous
"""Multi-head attention (B=2, N=2048, D=1024, H=16) on 8 Trainium2 cores.

Sharding: data-parallel over batch (cores 0-3 -> b=0, cores 4-7 -> b=1) and
tensor-parallel over heads (4 heads per core = 256 of 1024 QKV/O channels).
Each core computes its 4 heads' attention plus a partial output projection;
the host sums the 4 partials per batch and adds bo.

v2 pipeline (per core):
 - Input DMA spread over 4 engine queues; projections run chunk-major in a
   dedicated pre-phase PSUM pool so each weight-chunk matmul fires as soon
   as its xT d-chunk lands.
 - Attention processes a (pair, 512-query-chunk) block at a time.  Per
   k-iter the TWO heads of the pair run their scores matmuls CONCURRENTLY
   in disjoint PE row-groups (K=64 each, tile_position (0,0)/(64,0)) into
   the two halves of one [128,1024] PSUM tile; a single FD=1024 exp on
   ScalarE covers both heads; PV (M=65 with the ones/denominator column)
   runs per head with a one-iter lag like the baseline.
 - Normalization reads PSUM directly: reciprocal_approx_fast on the den
   row, GPSIMD partition_broadcast, one tensor_mul into at.
 - qc-outer / pair-inner block order lets oproj units for query chunk qc
   run as PE filler work two blocks later; only the last chunk's oproj
   trails the attention.
"""

import numpy as np

import concourse.bass as bass
import concourse.bacc as bacc
import concourse.tile as tile
from concourse import mybir
from concourse.bass_utils import run_bass_kernel_spmd

F32 = mybir.dt.float32
BF16 = mybir.dt.bfloat16
AF = mybir.ActivationFunctionType

B, N, D, H, HD = 2, 2048, 1024, 16, 64
E = 256            # channels per core (4 heads * 64)
DC = D // 128      # 8 contraction chunks for projections
NB = N // 128      # 16 token blocks / k chunks
QC = 512           # query chunk
NQC = N // QC      # 4 query chunks
SCALE = 1.0 / np.sqrt(HD)
DT = BF16


def _emit(nc):
    xT = nc.dram_tensor("xT", [D, N], DT, kind="ExternalInput")
    wqT = nc.dram_tensor("wqT", [D, E], DT, kind="ExternalInput")
    wkT = nc.dram_tensor("wkT", [D, E], DT, kind="ExternalInput")
    wvT = nc.dram_tensor("wvT", [D, E], DT, kind="ExternalInput")
    woT = nc.dram_tensor("woT", [E, D], DT, kind="ExternalInput")
    bq2 = nc.dram_tensor("bq2", [128, 2], F32, kind="ExternalInput")
    bk2 = nc.dram_tensor("bk2", [128, 2], F32, kind="ExternalInput")
    bv1 = nc.dram_tensor("bv1", [E], F32, kind="ExternalInput")
    vones = nc.dram_tensor("vones", [128, NB, 4], DT, kind="ExternalInput")
    out = nc.dram_tensor("out", [N, D], F32, kind="ExternalOutput")

    with tile.TileContext(nc) as tc:
        with tc.tile_pool(name="per", bufs=1) as per, \
             tc.tile_pool(name="wp", bufs=12) as wp, \
             tc.tile_pool(name="dn", bufs=2) as dn, \
             tc.tile_pool(name="up", bufs=2) as up, \
             tc.tile_pool(name="op", bufs=4) as op:

            # ---- persistent SBUF tiles ----
            # xt/wq/wk/wv are PER-CHUNK tiles: slices of one big tile would
            # serialize their input DMAs (tile-granular write tracking).
            xt = [per.tile([128, N], DT, name=f"xt{i}") for i in range(DC)]
            wq = [per.tile([128, E], DT, name=f"wq{i}") for i in range(DC)]
            wk = [per.tile([128, E], DT, name=f"wk{i}") for i in range(DC)]
            wv = [per.tile([128, E], DT, name=f"wv{i}") for i in range(DC)]
            wo = per.tile([128, 2, D], DT)            # WoT (e-chunk)
            qt = per.tile([128, 2, N], DT)            # Q^T: (pair, tokens)
            kt = per.tile([128, 2, N], DT)
            vp = per.tile([128, NB, 4, 128], DT)      # V natural + ones col
            at = per.tile([128, 2, N], DT)            # attn^T normalized
            bqs = per.tile([128, 2], F32)
            bks = per.tile([128, 2], F32)
            bvb = per.tile([128, E], F32)

            # ---- input DMA: per-d-chunk rounds over 3 queues ----
            # Each chunk's small W slices load ahead of its big xT slice on
            # the same queue so the chunk-major pre-phase can start as soon
            # as a whole d-chunk has landed.
            qs = [nc.sync, nc.scalar, nc.gpsimd]
            for dc in range(DC):
                q = qs[dc % 3]
                q.dma_start(out=wk[dc], in_=wkT[dc * 128:(dc + 1) * 128, :])
                q.dma_start(out=wq[dc], in_=wqT[dc * 128:(dc + 1) * 128, :])
                q.dma_start(out=wv[dc], in_=wvT[dc * 128:(dc + 1) * 128, :])
                q.dma_start(out=xt[dc], in_=xT[dc * 128:(dc + 1) * 128, :])
            nc.sync.dma_start(out=bqs, in_=bq2[:, :])
            nc.sync.dma_start(out=bks, in_=bk2[:, :])
            bv_ap = bv1[:]
            nc.gpsimd.dma_start(
                out=bvb,
                in_=bass.AP(tensor=bv_ap.tensor, offset=0, ap=[[0, 128], [1, E]]),
            )
            nc.scalar.dma_start(out=vp[:, :, :, HD:HD + 1],
                                in_=vones[:, :, :].rearrange(
                                    "p a (b o) -> p a b o", o=1))
            for ec in range(2):
                nc.gpsimd.dma_start(out=wo[:, ec, :],
                                    in_=woT[ec * 128:(ec + 1) * 128, :])

            # ---- pre-phase: warmup + chunk-major first projections ----
            # K0 g0-3, Q0 g0, V nb0-1 accumulate concurrently in a dedicated
            # PSUM pool (7 banks); each d-chunk's matmuls fire as the chunk
            # arrives from HBM.
            with tc.tile_pool(name="pre", bufs=1, space="PSUM") as pre:
                wu = per.tile([64, 512], DT)
                nc.vector.memset(wu, 0.0)
                wps = pre.tile([64, 512], F32, tag="p7", name="wps")
                for i in range(10):
                    nc.tensor.matmul(wps[:, :], wu[:, 0:64], wu[:, :],
                                     start=True, stop=True)

                pk = [pre.tile([128, 512], F32, tag=f"p{g}", name=f"pk{g}")
                      for g in range(4)]
                pq0 = pre.tile([128, 512], F32, tag="p4", name="pq0")
                pv01 = [pre.tile([128, E], F32, tag=f"p{5 + i}", name=f"pv{i}")
                        for i in range(2)]
                for dc in range(DC):
                    for g in range(4):
                        nc.tensor.matmul(
                            pk[g], wk[dc][:, 0:128],
                            xt[dc][:, g * 512:(g + 1) * 512],
                            start=(dc == 0), stop=(dc == DC - 1))
                    nc.tensor.matmul(
                        pq0, wq[dc][:, 0:128], xt[dc][:, 0:512],
                        start=(dc == 0), stop=(dc == DC - 1))
                    for i in range(2):
                        nc.tensor.matmul(
                            pv01[i], xt[dc][:, i * 128:(i + 1) * 128],
                            wv[dc],
                            start=(dc == 0), stop=(dc == DC - 1))
                for g in range(4):
                    nc.vector.tensor_scalar_add(
                        kt[:, 0, g * 512:(g + 1) * 512], pk[g], bks[:, 0:1])
                nc.vector.tensor_scalar_add(qt[:, 0, 0:512], pq0, bqs[:, 0:1])
                for i in range(2):
                    nc.vector.tensor_add(
                        vp[:, i, :, 0:HD],
                        pv01[i].rearrange("p (h d) -> p h d", h=4),
                        bvb.rearrange("p (h d) -> p h d", h=4))

            with tc.tile_pool(name="ps", bufs=1, space="PSUM") as ps:
                pj_n = [0]

                def pj_tag():
                    pj_n[0] += 1
                    return ("pjA", "pjB")[pj_n[0] % 2]

                # ---- filler units (1-bank psum groups on pj tags) ----
                # K/Q projection groups run at boosted priority: they feed
                # the NEXT block's scores and must not queue behind the
                # V-projection / PV backlog.
                def proj_group(wsb, dst, bias, pair, n4):
                    def emit():
                        with tc.high_priority(offset=1 << 19):
                            pt = ps.tile([128, 512], F32, tag=pj_tag(),
                                         name="ppj")
                            for dc in range(DC):
                                nc.tensor.matmul(
                                    pt[:, :],
                                    wsb[dc][:, pair * 128:(pair + 1) * 128],
                                    xt[dc][:, n4 * 512:(n4 + 1) * 512],
                                    start=(dc == 0), stop=(dc == DC - 1))
                            nc.vector.tensor_scalar_add(
                                dst[:, pair, n4 * 512:(n4 + 1) * 512], pt[:, :],
                                bias[:, pair:pair + 1])
                    return emit

                def vnat_group(nb):
                    def emit():
                        pt = ps.tile([128, E], F32, tag=pj_tag(), name="pvn")
                        for dc in range(DC):
                            nc.tensor.matmul(
                                pt[:, :],
                                xt[dc][:, nb * 128:(nb + 1) * 128],
                                wv[dc],
                                start=(dc == 0), stop=(dc == DC - 1))
                        nc.vector.tensor_add(
                            vp[:, nb, :, 0:HD],
                            pt.rearrange("p (h d) -> p h d", h=4),
                            bvb.rearrange("p (h d) -> p h d", h=4))
                    return emit

                o_n = [0]

                def oproj_unit(nb, evict="dve"):
                    # both D-halves of a 128-token block -> one contiguous
                    # [128, 1024] row store
                    def emit():
                        ot = op.tile([128, 1024], F32, tag="ot", name="ot")
                        for half in range(2):
                            po = ps.tile([128, 512], F32, tag=pj_tag(),
                                         name="po")
                            for ec in range(2):
                                nc.tensor.matmul(
                                    po[:, :],
                                    at[:, ec, nb * 128:(nb + 1) * 128],
                                    wo[:, ec, half * 512:(half + 1) * 512],
                                    start=(ec == 0), stop=(ec == 1))
                            if evict == "dve":
                                nc.vector.tensor_copy(
                                    ot[:, half * 512:(half + 1) * 512], po)
                            else:
                                nc.scalar.copy(
                                    ot[:, half * 512:(half + 1) * 512], po)
                        o_n[0] += 1
                        qs[o_n[0] % 3].dma_start(
                            out=out[nb * 128:(nb + 1) * 128, :], in_=ot)
                    return emit

                # ---- one (pair, qc) attention block: 16 k-iters ----
                # Returns a `finish` closure (last PV pair + normalization);
                # the caller runs it after the NEXT block's first k-iter so
                # ScalarE never stalls across block boundaries.
                def attn_block(pair, qc, fillers, carry=None):
                    q0 = qc * QC
                    fi = 0
                    pvs = [ps.tile([HD + 1, QC], F32, tag=t, name=t)
                           for t in ("pvA", "pvB")]
                    wtiles = {}
                    for k in range(NB):
                        st = ps.tile([128, 1024], F32,
                                     tag=("s0", "s1")[k % 2], name="st")
                        # scores for both heads back-to-back at max priority
                        # so they sit adjacent in the PE queue and overlap in
                        # disjoint row-groups of the array.
                        with tc.high_priority(offset=1 << 20):
                            for hh in range(2):
                                p0 = hh * HD
                                nc.tensor.matmul(
                                    st[:, hh * QC:(hh + 1) * QC],
                                    kt[p0:p0 + HD, pair, k * 128:(k + 1) * 128],
                                    qt[p0:p0 + HD, pair, q0:q0 + QC],
                                    start=True, stop=True,
                                    tile_position=(p0, 0))
                        w = wp.tile([128, 1024], DT, tag="w", name="w")
                        nc.scalar.activation(w, st, AF.Exp, scale=SCALE)
                        wtiles[k] = w
                        if k == 0 and carry is not None:
                            carry()
                            carry = None
                        while fi < (k + 1) * len(fillers) // NB:
                            fillers[fi]()
                            fi += 1
                        if k > 0:
                            wprev = wtiles.pop(k - 1)
                            for hh in range(2):
                                nc.tensor.matmul(
                                    pvs[hh][:, :],
                                    vp[:, k - 1, 2 * pair + hh, 0:HD + 1],
                                    wprev[:, hh * QC:(hh + 1) * QC],
                                    start=(k - 1 == 0), stop=False)
                    wlast = wtiles.pop(NB - 1)

                    def finish():
                        for hh in range(2):
                            nc.tensor.matmul(
                                pvs[hh][:, :],
                                vp[:, NB - 1, 2 * pair + hh, 0:HD + 1],
                                wlast[:, hh * QC:(hh + 1) * QC],
                                start=False, stop=True)
                        # normalize both heads (den/u staged through SBUF)
                        for hh in range(2):
                            p0 = hh * HD
                            den = dn.tile([1, QC], F32, tag="den", name="den")
                            rec = dn.tile([1, QC], F32, tag="rec", name="rec")
                            bcr = up.tile([HD, QC], F32, tag="bcr", name="bcr")
                            u = up.tile([HD, QC], F32, tag="u", name="u")
                            nc.vector.tensor_copy(den, pvs[hh][HD:HD + 1, :])
                            nc.vector.tensor_copy(u, pvs[hh][0:HD, :])
                            nc.vector.reciprocal_approx_fast(rec, den)
                            nc.gpsimd.partition_broadcast(bcr, rec[0:1, :])
                            nc.vector.tensor_mul(
                                at[p0:p0 + HD, pair, q0:q0 + QC], u, bcr)
                    return finish

                # ---- filler inventory ----
                V = [vnat_group(i) for i in range(NB)]
                K1 = [proj_group(wk, kt, bks, 1, g) for g in range(4)]
                Q0 = [proj_group(wq, qt, bqs, 0, g) for g in range(4)]
                Q1 = [proj_group(wq, qt, bqs, 1, g) for g in range(4)]
                O = [[oproj_unit(4 * qc + nb,
                                 evict=("dve" if qc < 3 else
                                        ("dve", "act")[nb % 2]))
                      for nb in range(4)] for qc in range(NQC)]

                # V[nb] feeds PV at iter nb of the qc=0 blocks; keep a
                # 2-iter lead.  K1 g must land before block (1,0) iter 4g.
                sched = [
                    (0, 0, [K1[0], V[2], V[3], V[4], V[5], V[6], V[7],
                            V[8], V[9], V[10], V[11], V[12], V[13],
                            V[14], V[15], Q1[0]]),
                    (1, 0, [K1[1], K1[2], K1[3], Q0[1]]),
                    (0, 1, [Q1[1]] + O[0][0:2]),
                    (1, 1, O[0][2:4] + [Q0[2]]),
                    (0, 2, [Q1[2]] + O[1][0:2]),
                    (1, 2, O[1][2:4] + [Q0[3]]),
                    (0, 3, [Q1[3]] + O[2][0:2]),
                    (1, 3, O[2][2:4]),
                ]
                fin = None
                for pair, qc, fillers in sched:
                    fin = attn_block(pair, qc, fillers, carry=fin)
                fin()
                for g in O[3]:
                    g()
    return nc


_CACHE = {}


def _build():
    if "nc" not in _CACHE:
        nc = bacc.Bacc("TRN2", target_bir_lowering=False, debug=False)
        _emit(nc)
        nc.compile()
        _CACHE["nc"] = nc
    return _CACHE["nc"]


def make_in_maps(x, Wq, bq, Wk, bk, Wv, bv, Wo, bo):
    import ml_dtypes
    f32 = np.float32
    bt = ml_dtypes.bfloat16
    ones_np = np.ones((128, NB, 4), bt)
    xTs = [np.ascontiguousarray(np.asarray(x[b], dtype=f32).T).astype(bt)
           for b in range(B)]
    in_maps = []
    for c in range(8):
        b, r0 = c // 4, (c % 4) * E
        rows = slice(r0, r0 + E)
        in_maps.append({
            "xT": xTs[b],
            "wqT": np.ascontiguousarray(np.asarray(Wq, f32)[rows].T).astype(bt),
            "wkT": np.ascontiguousarray(np.asarray(Wk, f32)[rows].T).astype(bt),
            "wvT": np.ascontiguousarray(np.asarray(Wv, f32)[rows].T).astype(bt),
            "woT": np.ascontiguousarray(np.asarray(Wo, f32)[:, rows].T).astype(bt),
            "bq2": np.ascontiguousarray(np.asarray(bq, f32)[rows].reshape(2, 128).T),
            "bk2": np.ascontiguousarray(np.asarray(bk, f32)[rows].reshape(2, 128).T),
            "bv1": np.ascontiguousarray(np.asarray(bv, f32)[rows]),
            "vones": ones_np,
        })
    return in_maps


def kernel(x, Wq, bq, Wk, bk, Wv, bv, Wo, bo, _spmd_kwargs=None):
    nc = _build()
    in_maps = make_in_maps(x, Wq, bq, Wk, bk, Wv, bv, Wo, bo)
    res = run_bass_kernel_spmd(nc, in_maps, core_ids=list(range(8)),
                               **(_spmd_kwargs or {}))
    parts = np.stack([res.results[c]["out"] for c in range(8)])
    outv = parts.reshape(B, 4, N, D).sum(axis=1) + np.asarray(bo, np.float32)
    if _spmd_kwargs:
        _CACHE["last_results"] = res
    return outv.astype(np.float32)
